# revision 5
# baseline (speedup 1.0000x reference)
"""Trainium2 Bass kernel for nn_BasicAttention (B=8, C=1024, L=2048, A=128).

Sharding: data-parallel over batch B — one example per NeuronCore, no
collectives.

Math (per example), using associativity to avoid any on-device transpose:
    keys    = Wk @ x + bk                      [A, L]
    queries = Wq @ x + bq                      [A, L]
    V       = keys^T @ queries                 [L, L]
    E       = exp(V / (L/2))   (raw exp; logits are ~1e-2 so no max-sub)
    S[l]    = sum_m E[l, m]
    yT      = x^T @ Wp^T       (= (Wp @ x)^T)  [L, C]
    out     = (yT / S)^T @ E + bp              [C, L]

The PE convention matmul(out, lhsT, rhs) = lhsT.T @ rhs with the
contraction on the partition dim lets every GEMM run without transposing
activations: host passes Wk^T/Wq^T/Wp^T, x tiles serve directly as lhsT
for yT, keys serve directly as lhsT for V, and yT serves directly as
lhsT for the final GEMM. E is staged through DRAM between the values
phase (row-major over l) and the final phase (column-chunk-major over m).

All matmuls run in float32r (~1e-4 rel err, full PE rate at N=512).
"""

import os
import sys

for _p in ("/opt/trn_rl_repo", "/root/.axon_site/_ro/trn_rl_repo"):
    if os.path.isdir(_p) and _p not in sys.path:
        sys.path.insert(0, _p)

import numpy as np
from contextlib import ExitStack

from concourse import bacc, mybir, tile
from concourse.bass_utils import run_bass_kernel_spmd

P = 128
B, C, L, A = 8, 1024, 2048, 128
NC_TILES = C // P          # 8 c-tiles
NL_TILES = L // P          # 16 l-tiles
ND_TILES = C // P          # 8 d-tiles (output channel tiles)
NCHUNK = 512               # fp32r moving-operand max
NMCH = L // NCHUNK         # 4 m-chunks

F32 = mybir.dt.float32
F32R = mybir.dt.float32r
AF = mybir.ActivationFunctionType

_NC_CACHE = {}


def build_nc(rep: int = 1):
    """Build the per-core Bass module. rep>1 repeats the whole compute for
    steady-state timing (outputs of extra reps overwrite)."""
    nc = bacc.Bacc(None, target_bir_lowering=False)

    x_d = nc.declare_dram_parameter("x", [C, L], F32R, isOutput=False)
    wkT_d = nc.declare_dram_parameter("wkT", [C, A], F32R, isOutput=False)
    wqT_d = nc.declare_dram_parameter("wqT", [C, A], F32R, isOutput=False)
    wpT_d = nc.declare_dram_parameter("wpT", [C, C], F32R, isOutput=False)
    bk_d = nc.declare_dram_parameter("bk", [A], F32, isOutput=False)
    bq_d = nc.declare_dram_parameter("bq", [A], F32, isOutput=False)
    bp_d = nc.declare_dram_parameter("bp", [ND_TILES, P], F32, isOutput=False)
    out_d = nc.declare_dram_parameter("out", [C, L], F32, isOutput=True)

    with tile.TileContext(nc) as tc, ExitStack() as octx:
        const = octx.enter_context(tc.tile_pool(name="const", bufs=1))
        xw = octx.enter_context(tc.tile_pool(name="xw", bufs=1))
        ytp = octx.enter_context(tc.tile_pool(name="ytp", bufs=1))
        dram = octx.enter_context(tc.tile_pool(name="dram", bufs=1, space="DRAM"))

        # ---- persistent SBUF loads ----
        wk_sb = const.tile([P, NC_TILES, A], F32R)
        wq_sb = const.tile([P, NC_TILES, A], F32R)
        bk_sb = const.tile([P, 1], F32)
        bq_sb = const.tile([P, 1], F32)
        bp_sb = const.tile([P, ND_TILES], F32)
        nc.sync.dma_start(out=wk_sb[:], in_=wkT_d.rearrange("(n p) a -> p n a", p=P))
        nc.sync.dma_start(out=wq_sb[:], in_=wqT_d.rearrange("(n p) a -> p n a", p=P))
        nc.sync.dma_start(out=bk_sb[:], in_=bk_d.rearrange("(p o) -> p o", o=1))
        nc.sync.dma_start(out=bq_sb[:], in_=bq_d.rearrange("(p o) -> p o", o=1))
        nc.sync.dma_start(out=bp_sb[:], in_=bp_d.rearrange("n p -> p n"))

        x_sb = xw.tile([P, NC_TILES, L], F32R)
        wp_sb = xw.tile([P, NC_TILES, C], F32R)
        for c in range(NC_TILES):
            nc.sync.dma_start(out=x_sb[:, c, :],
                              in_=x_d.rearrange("(n p) l -> p n l", p=P)[:, c, :])
            nc.sync.dma_start(out=wp_sb[:, c, :],
                              in_=wpT_d.rearrange("(n p) d -> p n d", p=P)[:, c, :])

        yt_sb = ytp.tile([P, NL_TILES, C], F32R)
        a_dram = dram.tile([NL_TILES, P, L], F32R)

        sml = octx.enter_context(tc.tile_pool(name="sml", bufs=1))
        s_all = sml.tile([P, NL_TILES], F32)
        rs_all = sml.tile([P, NL_TILES], F32)

        for _r in range(rep):
            # ================= Phase 1: keys / queries =================
            kq = tc.alloc_tile_pool(name="kq", bufs=1)
            ps1 = tc.alloc_tile_pool(name="ps1", bufs=4, space="PSUM")
            keys_sb = kq.tile([P, L], F32R)
            quer_sb = kq.tile([P, L], F32R)
            for w_sb, b_sb, o_sb in ((wk_sb, bk_sb, keys_sb),
                                     (wq_sb, bq_sb, quer_sb)):
                acc = [ps1.tile([P, NCHUNK], F32, tag="ps1", name=f"acc1_{_}") for _ in range(NMCH)]
                for c in range(NC_TILES):
                    for j in range(NMCH):
                        nc.tensor.matmul(
                            out=acc[j][:],
                            lhsT=w_sb[:, c, :],
                            rhs=x_sb[:, c, j * NCHUNK:(j + 1) * NCHUNK],
                            start=(c == 0), stop=(c == NC_TILES - 1))
                for j in range(NMCH):
                    nc.scalar.activation(
                        o_sb[:, j * NCHUNK:(j + 1) * NCHUNK], acc[j][:],
                        AF.Identity, bias=b_sb[:])
            ps1.release()

            # ============ Phase 2: V = keys^T @ queries; exp; S ============
            expp = tc.alloc_tile_pool(name="expp", bufs=2)
            ps2 = tc.alloc_tile_pool(name="ps2", bufs=2, space="PSUM")
            for l in range(NL_TILES):
                vps = ps2.tile([P, L], F32, tag="ps2")
                for j in range(NMCH):
                    nc.tensor.matmul(
                        out=vps[:, j * NCHUNK:(j + 1) * NCHUNK],
                        lhsT=keys_sb[:, l * P:(l + 1) * P],
                        rhs=quer_sb[:, j * NCHUNK:(j + 1) * NCHUNK],
                        start=True, stop=True)
                e_sb = expp.tile([P, L], F32R, tag="exp")
                nc.scalar.activation(e_sb[:], vps[:], AF.Exp,
                                     scale=2.0 / L,
                                     accum_out=s_all[:, l:l + 1])
                nc.sync.dma_start(out=a_dram[l], in_=e_sb[:])
            nc.vector.reciprocal(out=rs_all[:], in_=s_all[:])
            ps2.release()
            expp.release()
            kq.release()

            # ================= Phase 3: yT = x^T @ Wp^T, scaled by 1/S =====
            ps3 = tc.alloc_tile_pool(name="ps3", bufs=4, space="PSUM")
            for l in range(NL_TILES):
                for dc in range(C // NCHUNK):
                    acc = ps3.tile([P, NCHUNK], F32, tag="ps3")
                    for c in range(NC_TILES):
                        nc.tensor.matmul(
                            out=acc[:],
                            lhsT=x_sb[:, c, l * P:(l + 1) * P],
                            rhs=wp_sb[:, c, dc * NCHUNK:(dc + 1) * NCHUNK],
                            start=(c == 0), stop=(c == NC_TILES - 1))
                    nc.scalar.activation(
                        yt_sb[:, l, dc * NCHUNK:(dc + 1) * NCHUNK], acc[:],
                        AF.Copy, scale=rs_all[:, l:l + 1])
            ps3.release()

            # ================= Phase 4: out = yTs^T @ E + bp =================
            with ExitStack() as ctx:
                ain = ctx.enter_context(tc.tile_pool(name="ain", bufs=6))
                outp = ctx.enter_context(tc.tile_pool(name="outp", bufs=4))
                ps4 = ctx.enter_context(
                    tc.tile_pool(name="ps4", bufs=8, space="PSUM"))
                for j in range(NMCH):
                    acc = [ps4.tile([P, NCHUNK], F32, tag="ps4", name=f"acc4_{_}")
                           for _ in range(ND_TILES)]
                    for l in range(NL_TILES):
                        a_t = ain.tile([P, NCHUNK], F32R, tag="ain")
                        nc.sync.dma_start(
                            out=a_t[:],
                            in_=a_dram[l][:, j * NCHUNK:(j + 1) * NCHUNK])
                        for d in range(ND_TILES):
                            nc.tensor.matmul(
                                out=acc[d][:],
                                lhsT=yt_sb[:, l, d * P:(d + 1) * P],
                                rhs=a_t[:],
                                start=(l == 0), stop=(l == NL_TILES - 1))
                    for d in range(ND_TILES):
                        o_sb = outp.tile([P, NCHUNK], F32, tag="out")
                        nc.vector.tensor_scalar_add(
                            out=o_sb[:], in0=acc[d][:], scalar1=bp_sb[:, d:d + 1])
                        nc.sync.dma_start(
                            out=out_d.rearrange("(n p) l -> p n l", p=P)
                                [:, d, j * NCHUNK:(j + 1) * NCHUNK],
                            in_=o_sb[:])

    nc.compile()
    return nc


def _get_nc(rep: int = 1):
    if rep not in _NC_CACHE:
        _NC_CACHE[rep] = build_nc(rep)
    return _NC_CACHE[rep]


def make_in_maps(x, Wk, bk, Wq, bq, Wp, bp):
    wkT = np.ascontiguousarray(Wk.T).astype(np.float32)
    wqT = np.ascontiguousarray(Wq.T).astype(np.float32)
    wpT = np.ascontiguousarray(Wp.T).astype(np.float32)
    bp2 = np.ascontiguousarray(bp.reshape(ND_TILES, P)).astype(np.float32)
    return [{
        "x": np.ascontiguousarray(x[b]).astype(np.float32),
        "wkT": wkT, "wqT": wqT, "wpT": wpT,
        "bk": bk.astype(np.float32), "bq": bq.astype(np.float32), "bp": bp2,
    } for b in range(B)]


def kernel(x, Wk, bk, Wq, bq, Wp, bp):
    nc = _get_nc(1)
    in_maps = make_in_maps(x, Wk, bk, Wq, bq, Wp, bp)
    res = run_bass_kernel_spmd(nc, in_maps, list(range(B)))
    return np.stack([res.results[b]["out"] for b in range(B)]).astype(np.float32)
